# revision 19
# baseline (speedup 1.0000x reference)
"""Trainium2 Bass kernel for nn_DecoderGravity (edge-list gravity decoder).

Computes, for each edge e with src s=idx[0,e], dst d=idx[1,e]:
    out[e] = x[d, 128] - l * log(sum_k (x[s,k]-x[d,k])^2 + 0.01)

The v1 kernel was bottlenecked by GPSIMD SWDGE descriptor generation for
dma_gather (~8ns/index, 160k indices/core = 1.3ms). This version halves
the descriptor stream and overlaps everything else under it:

  * dst side: ONE dma_gather stream from a "pair table" xp[25088, 512B]
    (fp8 features + fp16 mass for nodes 2r and 2r+1 in one row; index =
    dst//2 fits int16 without lo/hi bucketing). 86016 padded slots ->
    ~690us of Pool time, the kernel's roofline. Even/odd halves are
    blended at the r2/mass level with a host parity mask.
  * src side: NO dma_gather. Edges are sorted by src block (128 nodes)
    and packed into 128-edge chunks such that chunk c only draws from a
    static window of K=2 blocks. Host ships per-chunk one-hot planes
    (fp8; pure index metadata); the Tensor engine multiplies them with
    the fp8 node table x_sb [128, 391*128] to materialize gathered src
    rows in PSUM (edge-major), overlapped under the Pool roof.
  * r2 = reduce((s-d)^2): ACT copies PSUM->fp16, DVE subtracts, ACT
    squares, DVE reduces (fp16 accum; ~1% r2 error is far inside the
    gate) for both dst halves. Epilogue: out = m - l*ln(r2 + eps).
  * Emission is software-pipelined: produce(t) [oh DMA, matmuls,
    gather] is emitted before consume(t-1) so no engine head-of-line
    blocks the Pool descriptor stream.
"""

import numpy as np
import ml_dtypes

import concourse.bass as bass
import concourse.tile as tile
from concourse import bacc, mybir
from concourse.bass_utils import run_bass_kernel_spmd

# Problem constants (hardcoded per contract).
N = 50000
D = 129
DM = 128
E = 640000
NUM_CORES = 8
P = 128
EC = E // NUM_CORES          # 80000 edges per core
NPAD = 50176                 # N padded to 128*392
NPAIR = NPAD // 2            # pair-table rows
NB = 392                     # src blocks: node n in block n%392, row n//392
K = 2                        # block window size per chunk
C_DEF = 656                  # chunks of 128 edge slots (pad >= 2.5%)
KC = 16                      # chunks per gather tile (2048 slots)
TW = KC * P                  # slots per tile
EPS = 0.01

f32 = mybir.dt.float32
fp16 = mybir.dt.float16
fp8 = mybir.dt.float8e4
i16 = mybir.dt.int16
FP8NP = ml_dtypes.float8_e4m3


def _w_lo(C):
    alpha = NB / C
    return np.minimum((np.arange(C) * alpha).astype(int), NB - K)


def build_program(C=C_DEF):
    assert C % KC == 0
    ntiles = C // KC
    w_lo = _w_lo(C)
    nc = bacc.Bacc("TRN2", target_bir_lowering=False, debug=False,
                   num_devices=NUM_CORES)
    x_ap = nc.dram_tensor("xpad", [NPAD, D], f32, kind="ExternalInput").ap()
    d16_ap = nc.dram_tensor("dst16", [P, C * 8], i16,
                            kind="ExternalInput").ap()
    oh_ap = nc.dram_tensor("ohd", [P, ntiles * K * TW], fp8,
                           kind="ExternalInput").ap()
    par_ap = nc.dram_tensor("par", [P, C], fp16, kind="ExternalInput").ap()
    cst_ap = nc.dram_tensor("cst", [P, 2], fp16, kind="ExternalInput").ap()
    out_ap = nc.dram_tensor("out", [P, C], f32, kind="ExternalOutput").ap()

    xp = nc.dram_tensor("xp", [NPAIR, 512], fp8).ap()

    with tile.TileContext(nc) as tc:
        with (
            tc.tile_pool(name="xt", bufs=3) as xtp,
            tc.tile_pool(name="pair", bufs=2) as pairp,
            tc.tile_pool(name="xsb", bufs=1) as xsbp,
            tc.tile_pool(name="oh", bufs=2) as ohp,
            tc.tile_pool(name="ssb", bufs=2) as ssbp,
            tc.tile_pool(name="dq", bufs=2) as dqp,
            tc.tile_pool(name="sq", bufs=2) as sqp,
            tc.tile_pool(name="wide", bufs=1) as widep,
            tc.tile_pool(name="ps", bufs=2, space="PSUM") as psp,
        ):
            # ---- phase C: small loads (scalar-engine HWDGE ring so they
            # don't delay the build stream on the sync ring) -----------
            idx_sb = widep.tile([P, C * 8], i16, tag="idx")
            nc.scalar.dma_start(idx_sb[:], d16_ap[:])
            par_sb = widep.tile([P, C], fp16, tag="par")
            nc.scalar.dma_start(par_sb[:], par_ap[:])
            cst = widep.tile([P, 2], fp16, tag="cst")
            nc.scalar.dma_start(cst[:], cst_ap[:])

            r2e_w = widep.tile([P, C], fp16, tag="r2e")
            r2o_w = widep.tile([P, C], fp16, tag="r2o")
            m2_w = widep.tile([P, C * 2], fp16, tag="m2")
            tm_w = widep.tile([P, C], fp16, tag="tm")
            mm_w = widep.tile([P, C], fp16, tag="mm")
            outw = widep.tile([P, C], f32, tag="outw")
            m2v = m2_w.rearrange("p (c two) -> p c two", two=2)

            # ---- phase A: build pair table xp + x_sb in one x pass ----
            # x rows viewed as [p, 196 pairs, 2, 129]; xp as [p, 196, 512].
            # Node n = p*392 + c2 (c2 = 2c+t): src block = n%392 = c2,
            # position in block = n//392 = p. So the same tiles also fill
            # x_sb (strided over c2 parity) with no second x read.
            xv = x_ap[:].rearrange("(p c two) d -> p c (two d)", p=P, two=2)
            xpv = xp[:].rearrange("(p c) d -> p c d", p=P)
            x_sb = xsbp.tile([P, NB * DM], fp8, tag="xsb")
            xsv = x_sb.rearrange("p (c f) -> p c f", f=DM)
            CP = NPAIR // P  # 196
            for c0 in range(0, CP, 16):
                cw = min(16, CP - c0)
                t = xtp.tile([P, 4128], f32, tag="xt")
                tv = t.rearrange("p (c d) -> p c d", d=2 * D)
                nc.sync.dma_start(tv[:, :cw, :], xv[:, c0:c0 + cw, :])
                dt = pairp.tile([P, KC, 512], fp8, tag="pair")
                dt16 = dt.bitcast(fp16)
                nc.vector.tensor_copy(dt[:, :cw, 0:DM], tv[:, :cw, 0:DM])
                nc.vector.tensor_copy(dt16[:, :cw, 64:65],
                                      tv[:, :cw, DM:DM + 1])
                nc.vector.tensor_copy(dt[:, :cw, 256:256 + DM],
                                      tv[:, :cw, D:D + DM])
                nc.vector.tensor_copy(dt16[:, :cw, 65:66],
                                      tv[:, :cw, 2 * D - 1:2 * D])
                nc.sync.dma_start(xpv[:, c0:c0 + cw, :], dt[:, :cw, :])
                nc.vector.tensor_copy(xsv[:, 2 * c0:2 * (c0 + cw):2, :],
                                      tv[:, :cw, 0:DM])
                nc.vector.tensor_copy(xsv[:, 2 * c0 + 1:2 * (c0 + cw):2, :],
                                      tv[:, :cw, D:D + DM])

            # ---- phase D: software-pipelined main loop ---------------
            # gathers are paired (4096 idx) to amortize SWDGE overhead
            tiles = {}
            gtiles = {}

            def produce(t_i):
                if t_i % 2 == 0:
                    nt2 = min(2, ntiles - t_i)
                    isl = slice(t_i * P, (t_i + nt2) * P)
                    gt = pairp.tile([P, 2 * KC, 512], fp8, tag="gpair")
                    nc.gpsimd.dma_gather(gt[:, :nt2 * KC, :], xp[:],
                                         idx_sb[:, isl], nt2 * TW, nt2 * TW,
                                         512, single_packet=False)
                    gtiles[t_i // 2] = gt
                osl = slice(t_i * K * TW, (t_i + 1) * K * TW)
                oh = ohp.tile([P, K, TW], fp8, tag="oh")
                nc.sync.dma_start(oh.rearrange("p k w -> p (k w)"),
                                  oh_ap[:, osl])
                ps = psp.tile([P, TW], f32, tag="ps")
                for ch in range(KC):
                    g = t_i * KC + ch
                    w = int(w_lo[g])
                    cs = slice(ch * P, (ch + 1) * P)
                    for k in range(K):
                        nc.tensor.matmul(
                            ps[:, cs], oh[:, k:k + 1, cs],
                            xsv[:, w + k, :],
                            start=(k == 0), stop=(k == K - 1))
                tiles[t_i] = ps

            def consume(t_i):
                ps = tiles.pop(t_i)
                gt = gtiles[t_i // 2]
                h0 = (t_i % 2) * KC
                pt = gt[:, h0:h0 + KC, :]
                slc = slice(t_i * KC, (t_i + 1) * KC)
                sl2 = slice(t_i * KC * 2, (t_i + 1) * KC * 2)
                pt16 = gt.bitcast(fp16)[:, h0:h0 + KC, :]
                ssb = ssbp.tile([P, TW], fp16, tag="ssb")
                nc.scalar.activation(ssb[:], ps[:],
                                     mybir.ActivationFunctionType.Copy)
                sv = ssb.rearrange("p (c f) -> p c f", f=DM)
                de = dqp.tile([P, KC, DM], fp16, tag="dq")
                nc.vector.tensor_tensor(out=de[:], in0=sv[:],
                                        in1=pt[:, :, 0:DM],
                                        op=mybir.AluOpType.subtract)
                se = sqp.tile([P, KC, DM], fp16, tag="sq")
                nc.scalar.activation(se[:], de[:],
                                     mybir.ActivationFunctionType.Square)
                with nc.allow_low_precision("r2 fp16 accum: ~1% worst-case"):
                    nc.vector.tensor_reduce(r2e_w[:, slc], se[:],
                                            axis=mybir.AxisListType.X,
                                            op=mybir.AluOpType.add)
                do = dqp.tile([P, KC, DM], fp16, tag="dq")
                nc.vector.tensor_tensor(out=do[:], in0=sv[:],
                                        in1=pt[:, :, 256:256 + DM],
                                        op=mybir.AluOpType.subtract)
                so = sqp.tile([P, KC, DM], fp16, tag="sq")
                nc.scalar.activation(so[:], do[:],
                                     mybir.ActivationFunctionType.Square)
                with nc.allow_low_precision("r2 fp16 accum: ~1% worst-case"):
                    nc.vector.tensor_reduce(r2o_w[:, slc], so[:],
                                            axis=mybir.AxisListType.X,
                                            op=mybir.AluOpType.add)
                nc.scalar.activation(
                    m2_w[:, sl2].rearrange("p (c two) -> p c two", two=2),
                    pt16[:, :, 64:66], mybir.ActivationFunctionType.Copy)

            # epilogue over a column range (split so most of it overlaps
            # the tail of the gather stream):
            # r2 = r2e + par*(r2o - r2e); m likewise; out = m - l*ln(r2+eps)
            def epilogue(c0, c1):
                cs = slice(c0, c1)
                cs2 = slice(c0, c1)
                nc.vector.tensor_tensor(
                    out=r2o_w[:, cs], in0=r2o_w[:, cs], in1=r2e_w[:, cs],
                    op=mybir.AluOpType.subtract)
                nc.vector.tensor_tensor(
                    out=r2o_w[:, cs], in0=r2o_w[:, cs], in1=par_sb[:, cs],
                    op=mybir.AluOpType.mult)
                nc.vector.tensor_tensor(
                    out=r2e_w[:, cs], in0=r2e_w[:, cs], in1=r2o_w[:, cs],
                    op=mybir.AluOpType.add)
                nc.vector.tensor_tensor(
                    out=tm_w[:, cs].unsqueeze(2), in0=m2v[:, cs2, 1:2],
                    in1=m2v[:, cs2, 0:1], op=mybir.AluOpType.subtract)
                nc.vector.tensor_tensor(
                    out=tm_w[:, cs], in0=tm_w[:, cs], in1=par_sb[:, cs],
                    op=mybir.AluOpType.mult)
                nc.vector.tensor_tensor(
                    out=mm_w[:, cs].unsqueeze(2), in0=m2v[:, cs2, 0:1],
                    in1=tm_w[:, cs].unsqueeze(2), op=mybir.AluOpType.add)
                nc.scalar.activation(r2o_w[:, cs], r2e_w[:, cs],
                                     mybir.ActivationFunctionType.Ln,
                                     bias=cst[:, 1:2])
                nc.vector.scalar_tensor_tensor(
                    out=outw[:, cs], in0=r2o_w[:, cs], scalar=cst[:, 0:1],
                    in1=mm_w[:, cs],
                    op0=mybir.AluOpType.mult, op1=mybir.AluOpType.add)
                nc.sync.dma_start(out_ap[:, cs], outw[:, cs])

            ntl = ntiles
            for t_i in range(ntl + 1):
                if t_i < ntl:
                    produce(t_i)
                if t_i > 0:
                    consume(t_i - 1)
                    if t_i == ntl - 1:
                        # all but the last tile's columns: overlaps the
                        # final gather + consume
                        epilogue(0, (ntl - 1) * KC)
            epilogue((ntl - 1) * KC, C)

    nc.compile()
    return nc


_compiled = {}


def _get_compiled(C=C_DEF):
    if C not in _compiled:
        _compiled[C] = build_program(C)
    return _compiled[C]


def _pack_core(src, C):
    """Window-pack edges (sorted by src block) into C chunks of 128 slots.

    Returns slot2edge [C*128] int64 (-1 = pad). Raises OverflowError if C
    is too small.
    """
    w_lo = _w_lo(C)
    order = np.argsort(src % NB, kind="stable")
    blocks = (src[order] % NB).astype(np.int64)
    counts = np.bincount(blocks, minlength=NB)
    slot2edge = np.full(C * P, -1, np.int64)
    c = 0
    fill = 0
    pos = 0
    for b in range(NB):
        n = int(counts[b])
        while n > 0:
            while c < C and not (w_lo[c] <= b < w_lo[c] + K):
                c += 1
                fill = 0
            if c >= C:
                raise OverflowError(C)
            take = min(n, P - fill)
            s0 = c * P + fill
            slot2edge[s0:s0 + take] = order[pos:pos + take]
            pos += take
            n -= take
            fill += take
            if fill == P:
                c += 1
                fill = 0
                if c >= C and n > 0:
                    raise OverflowError(C)
    return slot2edge


def make_in_maps(x, edge_label_index, l_param, C=C_DEF):
    x = np.asarray(x, dtype=np.float32)
    x_pad = np.zeros((NPAD, D), np.float32)
    x_pad[:N] = x
    eli = np.asarray(edge_label_index)
    l = float(np.asarray(l_param).reshape(-1)[0])
    cstv = np.zeros((P, 2), np.float16)
    cstv[:, 0] = -l
    cstv[:, 1] = EPS
    w_lo = _w_lo(C)
    ntiles = C // KC
    chunk_of_slot = np.arange(C * P) // P

    in_maps = []
    slot_maps = []
    for core in range(NUM_CORES):
        sl = slice(core * EC, (core + 1) * EC)
        src = eli[0][sl].astype(np.int64)
        dst = eli[1][sl].astype(np.int64)
        s2e = _pack_core(src, C)
        real = s2e >= 0
        e = s2e[real]
        slots = np.arange(C * P)[real]
        vp = src[e] // NB                            # position in block
        vk = src[e] % NB - w_lo[chunk_of_slot[real]]  # [0, K)
        oh8 = np.zeros((P, ntiles, K, TW), np.uint8)
        oh8[vp, slots // TW, vk, slots % TW] = 0x38  # 1.0 in e4m3
        dstv = np.zeros(C * P, np.int16)
        dstv[real] = (dst[e] // 2).astype(np.int16)
        d16 = np.tile(dstv.reshape(C * P // 16, 16).T, (8, 1))
        par = np.zeros(C * P, np.float16)
        par[real] = (dst[e] % 2).astype(np.float16)
        in_maps.append({
            "xpad": x_pad,
            "dst16": np.ascontiguousarray(d16),
            "ohd": oh8.reshape(P, ntiles * K * TW).view(FP8NP),
            "par": np.ascontiguousarray(par.reshape(C, P).T),
            "cst": cstv,
        })
        slot_maps.append(s2e)
    return in_maps, slot_maps, C


def _unshard(results, slot_maps, C):
    out = np.empty(E, np.float32)
    for core in range(NUM_CORES):
        dev = results[core]["out"]          # [128, C]
        vals = dev.T.ravel()                # slot-major
        s2e = slot_maps[core]
        real = s2e >= 0
        core_out = np.empty(EC, np.float32)
        core_out[s2e[real]] = vals[real]
        out[core * EC:(core + 1) * EC] = core_out
    return out.reshape(E, 1)


def kernel(x, edge_label_index, l_param):
    C = C_DEF
    while True:
        try:
            in_maps, slot_maps, C = make_in_maps(
                x, edge_label_index, l_param, C)
            break
        except OverflowError:
            C += 2 * KC
    nc = _get_compiled(C)
    res = run_bass_kernel_spmd(nc, in_maps, list(range(NUM_CORES)))
    return _unshard(res.results, slot_maps, C)


# revision 20
# speedup vs baseline: 1.1509x; 1.1509x over previous
"""Trainium2 Bass kernel for nn_DecoderGravity (edge-list gravity decoder).

Computes, for each edge e with src s=idx[0,e], dst d=idx[1,e]:
    out[e] = x[d, 128] - l * log(sum_k (x[s,k]-x[d,k])^2 + 0.01)

The v1 kernel was bottlenecked by GPSIMD SWDGE descriptor generation for
dma_gather (~8ns/index, 160k indices/core = 1.3ms). This version halves
the descriptor stream and overlaps everything else under it:

  * dst side: ONE dma_gather stream from a "pair table" xp[25088, 512B]
    (fp8 features + fp16 mass for nodes 2r and 2r+1 in one row; index =
    dst//2 fits int16 without lo/hi bucketing). 86016 padded slots ->
    ~690us of Pool time, the kernel's roofline. Even/odd halves are
    blended at the r2/mass level with a host parity mask.
  * src side: NO dma_gather. Edges are sorted by src block (128 nodes)
    and packed into 128-edge chunks such that chunk c only draws from a
    static window of K=2 blocks. Host ships per-chunk one-hot planes
    (fp8; pure index metadata); the Tensor engine multiplies them with
    the fp8 node table x_sb [128, 391*128] to materialize gathered src
    rows in PSUM (edge-major), overlapped under the Pool roof.
  * r2 = reduce((s-d)^2): ACT copies PSUM->fp16, DVE subtracts, ACT
    squares, DVE reduces (fp16 accum; ~1% r2 error is far inside the
    gate) for both dst halves. Epilogue: out = m - l*ln(r2 + eps).
  * Emission is software-pipelined: produce(t) [oh DMA, matmuls,
    gather] is emitted before consume(t-1) so no engine head-of-line
    blocks the Pool descriptor stream.
"""

import numpy as np
import ml_dtypes

import concourse.bass as bass
import concourse.tile as tile
from concourse import bacc, mybir
from concourse.bass_utils import run_bass_kernel_spmd

# Problem constants (hardcoded per contract).
N = 50000
D = 129
DM = 128
E = 640000
NUM_CORES = 8
P = 128
EC = E // NUM_CORES          # 80000 edges per core
NPAD = 50176                 # N padded to 128*392
NPAIR = NPAD // 2            # pair-table rows
NB = 392                     # src blocks: node n in block n%392, row n//392
K = 2                        # block window size per chunk
C_DEF = 656                  # chunks of 128 edge slots (pad >= 2.5%)
KC = 16                      # chunks per gather tile (2048 slots)
TW = KC * P                  # slots per tile
EPS = 0.01

f32 = mybir.dt.float32
fp16 = mybir.dt.float16
fp8 = mybir.dt.float8e4
i16 = mybir.dt.int16
FP8NP = ml_dtypes.float8_e4m3


def _w_lo(C):
    alpha = NB / C
    return np.minimum((np.arange(C) * alpha).astype(int), NB - K)


def build_program(C=C_DEF):
    assert C % KC == 0
    ntiles = C // KC
    w_lo = _w_lo(C)
    nc = bacc.Bacc("TRN2", target_bir_lowering=False, debug=False,
                   num_devices=NUM_CORES)
    x_ap = nc.dram_tensor("xpad", [NPAD, D], f32, kind="ExternalInput").ap()
    d16_ap = nc.dram_tensor("dst16", [P, C * 8], i16,
                            kind="ExternalInput").ap()
    oh_ap = nc.dram_tensor("ohd", [P, ntiles * K * TW], fp8,
                           kind="ExternalInput").ap()
    par_ap = nc.dram_tensor("par", [P, C], fp16, kind="ExternalInput").ap()
    cst_ap = nc.dram_tensor("cst", [P, 2], fp16, kind="ExternalInput").ap()
    out_ap = nc.dram_tensor("out", [P, C], f32, kind="ExternalOutput").ap()

    xp = nc.dram_tensor("xp", [NPAIR, 512], fp8).ap()

    with tile.TileContext(nc) as tc:
        with (
            tc.tile_pool(name="xt", bufs=3) as xtp,
            tc.tile_pool(name="pair", bufs=2) as pairp,
            tc.tile_pool(name="xsb", bufs=1) as xsbp,
            tc.tile_pool(name="oh", bufs=2) as ohp,
            tc.tile_pool(name="ssb", bufs=2) as ssbp,
            tc.tile_pool(name="dq", bufs=2) as dqp,
            tc.tile_pool(name="sq", bufs=2) as sqp,
            tc.tile_pool(name="wide", bufs=1) as widep,
            tc.tile_pool(name="ps", bufs=2, space="PSUM") as psp,
        ):
            # ---- phase C: small loads (first: gathers dep on idx) ----
            idx_sb = widep.tile([P, C * 8], i16, tag="idx")
            nc.sync.dma_start(idx_sb[:], d16_ap[:])
            par_sb = widep.tile([P, C], fp16, tag="par")
            nc.sync.dma_start(par_sb[:], par_ap[:])
            cst = widep.tile([P, 2], fp16, tag="cst")
            nc.sync.dma_start(cst[:], cst_ap[:])

            r2e_w = widep.tile([P, C], fp16, tag="r2e")
            r2o_w = widep.tile([P, C], fp16, tag="r2o")
            m2_w = widep.tile([P, C * 2], fp16, tag="m2")
            tm_w = widep.tile([P, C], fp16, tag="tm")
            mm_w = widep.tile([P, C], fp16, tag="mm")
            outw = widep.tile([P, C], f32, tag="outw")
            m2v = m2_w.rearrange("p (c two) -> p c two", two=2)

            # ---- phase A: build pair table xp + x_sb in one x pass ----
            # x rows viewed as [p, 196 pairs, 2, 129]; xp as [p, 196, 512].
            # Node n = p*392 + c2 (c2 = 2c+t): src block = n%392 = c2,
            # position in block = n//392 = p. So the same tiles also fill
            # x_sb (strided over c2 parity) with no second x read.
            xv = x_ap[:].rearrange("(p c two) d -> p c (two d)", p=P, two=2)
            xpv = xp[:].rearrange("(p c) d -> p c d", p=P)
            x_sb = xsbp.tile([P, NB * DM], fp8, tag="xsb")
            xsv = x_sb.rearrange("p (c f) -> p c f", f=DM)
            CP = NPAIR // P  # 196
            for c0 in range(0, CP, 16):
                cw = min(16, CP - c0)
                t = xtp.tile([P, 4128], f32, tag="xt")
                tv = t.rearrange("p (c d) -> p c d", d=2 * D)
                nc.sync.dma_start(tv[:, :cw, :], xv[:, c0:c0 + cw, :])
                dt = pairp.tile([P, KC, 512], fp8, tag="pair")
                dt16 = dt.bitcast(fp16)
                nc.vector.tensor_copy(dt[:, :cw, 0:DM], tv[:, :cw, 0:DM])
                nc.vector.tensor_copy(dt16[:, :cw, 64:65],
                                      tv[:, :cw, DM:DM + 1])
                nc.vector.tensor_copy(dt[:, :cw, 256:256 + DM],
                                      tv[:, :cw, D:D + DM])
                nc.vector.tensor_copy(dt16[:, :cw, 65:66],
                                      tv[:, :cw, 2 * D - 1:2 * D])
                nc.sync.dma_start(xpv[:, c0:c0 + cw, :], dt[:, :cw, :])
                nc.vector.tensor_copy(xsv[:, 2 * c0:2 * (c0 + cw):2, :],
                                      tv[:, :cw, 0:DM])
                nc.vector.tensor_copy(xsv[:, 2 * c0 + 1:2 * (c0 + cw):2, :],
                                      tv[:, :cw, D:D + DM])

            # ---- phase D: software-pipelined main loop ---------------
            # gathers are paired (4096 idx) to amortize SWDGE overhead
            tiles = {}
            gtiles = {}

            def produce(t_i):
                if t_i % 2 == 0:
                    nt2 = min(2, ntiles - t_i)
                    isl = slice(t_i * P, (t_i + nt2) * P)
                    gt = pairp.tile([P, 2 * KC, 512], fp8, tag="gpair")
                    nc.gpsimd.dma_gather(gt[:, :nt2 * KC, :], xp[:],
                                         idx_sb[:, isl], nt2 * TW, nt2 * TW,
                                         512, single_packet=False)
                    gtiles[t_i // 2] = gt
                osl = slice(t_i * K * TW, (t_i + 1) * K * TW)
                oh = ohp.tile([P, K, TW], fp8, tag="oh")
                nc.sync.dma_start(oh.rearrange("p k w -> p (k w)"),
                                  oh_ap[:, osl])
                ps = psp.tile([P, TW], f32, tag="ps")
                for ch in range(KC):
                    g = t_i * KC + ch
                    w = int(w_lo[g])
                    cs = slice(ch * P, (ch + 1) * P)
                    for k in range(K):
                        nc.tensor.matmul(
                            ps[:, cs], oh[:, k:k + 1, cs],
                            xsv[:, w + k, :],
                            start=(k == 0), stop=(k == K - 1))
                tiles[t_i] = ps

            def consume(t_i):
                ps = tiles.pop(t_i)
                gt = gtiles[t_i // 2]
                h0 = (t_i % 2) * KC
                pt = gt[:, h0:h0 + KC, :]
                slc = slice(t_i * KC, (t_i + 1) * KC)
                sl2 = slice(t_i * KC * 2, (t_i + 1) * KC * 2)
                pt16 = gt.bitcast(fp16)[:, h0:h0 + KC, :]
                ssb = ssbp.tile([P, TW], fp16, tag="ssb")
                nc.scalar.activation(ssb[:], ps[:],
                                     mybir.ActivationFunctionType.Copy)
                sv = ssb.rearrange("p (c f) -> p c f", f=DM)
                de = dqp.tile([P, KC, DM], fp16, tag="dq")
                nc.vector.tensor_tensor(out=de[:], in0=sv[:],
                                        in1=pt[:, :, 0:DM],
                                        op=mybir.AluOpType.subtract)
                se = sqp.tile([P, KC, DM], fp16, tag="sq")
                nc.scalar.activation(se[:], de[:],
                                     mybir.ActivationFunctionType.Square)
                with nc.allow_low_precision("r2 fp16 accum: ~1% worst-case"):
                    nc.vector.tensor_reduce(r2e_w[:, slc], se[:],
                                            axis=mybir.AxisListType.X,
                                            op=mybir.AluOpType.add)
                do = dqp.tile([P, KC, DM], fp16, tag="dq")
                nc.vector.tensor_tensor(out=do[:], in0=sv[:],
                                        in1=pt[:, :, 256:256 + DM],
                                        op=mybir.AluOpType.subtract)
                so = sqp.tile([P, KC, DM], fp16, tag="sq")
                nc.scalar.activation(so[:], do[:],
                                     mybir.ActivationFunctionType.Square)
                with nc.allow_low_precision("r2 fp16 accum: ~1% worst-case"):
                    nc.vector.tensor_reduce(r2o_w[:, slc], so[:],
                                            axis=mybir.AxisListType.X,
                                            op=mybir.AluOpType.add)
                nc.scalar.activation(
                    m2_w[:, sl2].rearrange("p (c two) -> p c two", two=2),
                    pt16[:, :, 64:66], mybir.ActivationFunctionType.Copy)

            # epilogue over a column range (split so most of it overlaps
            # the tail of the gather stream):
            # r2 = r2e + par*(r2o - r2e); m likewise; out = m - l*ln(r2+eps)
            def epilogue(c0, c1):
                cs = slice(c0, c1)
                cs2 = slice(c0, c1)
                nc.vector.tensor_tensor(
                    out=r2o_w[:, cs], in0=r2o_w[:, cs], in1=r2e_w[:, cs],
                    op=mybir.AluOpType.subtract)
                nc.vector.tensor_tensor(
                    out=r2o_w[:, cs], in0=r2o_w[:, cs], in1=par_sb[:, cs],
                    op=mybir.AluOpType.mult)
                nc.vector.tensor_tensor(
                    out=r2e_w[:, cs], in0=r2e_w[:, cs], in1=r2o_w[:, cs],
                    op=mybir.AluOpType.add)
                nc.vector.tensor_tensor(
                    out=tm_w[:, cs].unsqueeze(2), in0=m2v[:, cs2, 1:2],
                    in1=m2v[:, cs2, 0:1], op=mybir.AluOpType.subtract)
                nc.vector.tensor_tensor(
                    out=tm_w[:, cs], in0=tm_w[:, cs], in1=par_sb[:, cs],
                    op=mybir.AluOpType.mult)
                nc.vector.tensor_tensor(
                    out=mm_w[:, cs].unsqueeze(2), in0=m2v[:, cs2, 0:1],
                    in1=tm_w[:, cs].unsqueeze(2), op=mybir.AluOpType.add)
                nc.scalar.activation(r2o_w[:, cs], r2e_w[:, cs],
                                     mybir.ActivationFunctionType.Ln,
                                     bias=cst[:, 1:2])
                nc.vector.scalar_tensor_tensor(
                    out=outw[:, cs], in0=r2o_w[:, cs], scalar=cst[:, 0:1],
                    in1=mm_w[:, cs],
                    op0=mybir.AluOpType.mult, op1=mybir.AluOpType.add)
                nc.sync.dma_start(out_ap[:, cs], outw[:, cs])

            ntl = ntiles
            for t_i in range(ntl + 1):
                if t_i < ntl:
                    produce(t_i)
                if t_i > 0:
                    consume(t_i - 1)
                    if t_i == ntl - 1:
                        # all but the last tile's columns: overlaps the
                        # final gather + consume
                        epilogue(0, (ntl - 1) * KC)
            epilogue((ntl - 1) * KC, C)

    nc.compile()
    return nc


_compiled = {}


def _get_compiled(C=C_DEF):
    if C not in _compiled:
        _compiled[C] = build_program(C)
    return _compiled[C]


def _pack_core(src, C):
    """Window-pack edges (sorted by src block) into C chunks of 128 slots.

    Returns slot2edge [C*128] int64 (-1 = pad). Raises OverflowError if C
    is too small.
    """
    w_lo = _w_lo(C)
    order = np.argsort(src % NB, kind="stable")
    blocks = (src[order] % NB).astype(np.int64)
    counts = np.bincount(blocks, minlength=NB)
    slot2edge = np.full(C * P, -1, np.int64)
    c = 0
    fill = 0
    pos = 0
    for b in range(NB):
        n = int(counts[b])
        while n > 0:
            while c < C and not (w_lo[c] <= b < w_lo[c] + K):
                c += 1
                fill = 0
            if c >= C:
                raise OverflowError(C)
            take = min(n, P - fill)
            s0 = c * P + fill
            slot2edge[s0:s0 + take] = order[pos:pos + take]
            pos += take
            n -= take
            fill += take
            if fill == P:
                c += 1
                fill = 0
                if c >= C and n > 0:
                    raise OverflowError(C)
    return slot2edge


def make_in_maps(x, edge_label_index, l_param, C=C_DEF):
    x = np.asarray(x, dtype=np.float32)
    x_pad = np.zeros((NPAD, D), np.float32)
    x_pad[:N] = x
    eli = np.asarray(edge_label_index)
    l = float(np.asarray(l_param).reshape(-1)[0])
    cstv = np.zeros((P, 2), np.float16)
    cstv[:, 0] = -l
    cstv[:, 1] = EPS
    w_lo = _w_lo(C)
    ntiles = C // KC
    chunk_of_slot = np.arange(C * P) // P

    in_maps = []
    slot_maps = []
    for core in range(NUM_CORES):
        sl = slice(core * EC, (core + 1) * EC)
        src = eli[0][sl].astype(np.int64)
        dst = eli[1][sl].astype(np.int64)
        s2e = _pack_core(src, C)
        real = s2e >= 0
        e = s2e[real]
        slots = np.arange(C * P)[real]
        vp = src[e] // NB                            # position in block
        vk = src[e] % NB - w_lo[chunk_of_slot[real]]  # [0, K)
        oh8 = np.zeros((P, ntiles, K, TW), np.uint8)
        oh8[vp, slots // TW, vk, slots % TW] = 0x38  # 1.0 in e4m3
        dstv = np.zeros(C * P, np.int16)
        dstv[real] = (dst[e] // 2).astype(np.int16)
        d16 = np.tile(dstv.reshape(C * P // 16, 16).T, (8, 1))
        par = np.zeros(C * P, np.float16)
        par[real] = (dst[e] % 2).astype(np.float16)
        in_maps.append({
            "xpad": x_pad,
            "dst16": np.ascontiguousarray(d16),
            "ohd": oh8.reshape(P, ntiles * K * TW).view(FP8NP),
            "par": np.ascontiguousarray(par.reshape(C, P).T),
            "cst": cstv,
        })
        slot_maps.append(s2e)
    return in_maps, slot_maps, C


def _unshard(results, slot_maps, C):
    out = np.empty(E, np.float32)
    for core in range(NUM_CORES):
        dev = results[core]["out"]          # [128, C]
        vals = dev.T.ravel()                # slot-major
        s2e = slot_maps[core]
        real = s2e >= 0
        core_out = np.empty(EC, np.float32)
        core_out[s2e[real]] = vals[real]
        out[core * EC:(core + 1) * EC] = core_out
    return out.reshape(E, 1)


def kernel(x, edge_label_index, l_param):
    C = C_DEF
    while True:
        try:
            in_maps, slot_maps, C = make_in_maps(
                x, edge_label_index, l_param, C)
            break
        except OverflowError:
            C += 2 * KC
    nc = _get_compiled(C)
    res = run_bass_kernel_spmd(nc, in_maps, list(range(NUM_CORES)))
    return _unshard(res.results, slot_maps, C)


# revision 23
# speedup vs baseline: 1.1684x; 1.0152x over previous
"""Trainium2 Bass kernel for nn_DecoderGravity (edge-list gravity decoder).

Computes, for each edge e with src s=idx[0,e], dst d=idx[1,e]:
    out[e] = x[d, 128] - l * log(sum_k (x[s,k]-x[d,k])^2 + 0.01)

The v1 kernel was bottlenecked by GPSIMD SWDGE descriptor generation for
dma_gather (~8ns/index, 160k indices/core = 1.3ms). This version halves
the descriptor stream and overlaps everything else under it:

  * dst side: ONE dma_gather stream from a "pair table" xp[25088, 512B]
    (fp8 features + fp16 mass for nodes 2r and 2r+1 in one row; index =
    dst//2 fits int16 without lo/hi bucketing). 86016 padded slots ->
    ~690us of Pool time, the kernel's roofline. Even/odd halves are
    blended at the r2/mass level with a host parity mask.
  * src side: NO dma_gather. Edges are sorted by src block (128 nodes)
    and packed into 128-edge chunks such that chunk c only draws from a
    static window of K=2 blocks. Host ships per-chunk one-hot planes
    (fp8; pure index metadata); the Tensor engine multiplies them with
    the fp8 node table x_sb [128, 391*128] to materialize gathered src
    rows in PSUM (edge-major), overlapped under the Pool roof.
  * r2 = reduce((s-d)^2): ACT copies PSUM->fp16, DVE subtracts, ACT
    squares, DVE reduces (fp16 accum; ~1% r2 error is far inside the
    gate) for both dst halves. Epilogue: out = m - l*ln(r2 + eps).
  * Emission is software-pipelined: produce(t) [oh DMA, matmuls,
    gather] is emitted before consume(t-1) so no engine head-of-line
    blocks the Pool descriptor stream.
"""

import numpy as np
import ml_dtypes

import concourse.bass as bass
import concourse.tile as tile
from concourse import bacc, mybir
from concourse.bass_utils import run_bass_kernel_spmd

# Problem constants (hardcoded per contract).
N = 50000
D = 129
DM = 128
E = 640000
NUM_CORES = 8
P = 128
EC = E // NUM_CORES          # 80000 edges per core
NPAD = 50176                 # N padded to 128*392
NPAIR = NPAD // 2            # pair-table rows
NB = 392                     # src blocks: node n in block n%392, row n//392
K = 2                        # block window size per chunk
C_DEF = 656                  # chunks of 128 edge slots (pad >= 2.5%)
KC = 16                      # chunks per gather tile (2048 slots)
TW = KC * P                  # slots per tile
EPS = 0.01

f32 = mybir.dt.float32
fp16 = mybir.dt.float16
fp8 = mybir.dt.float8e4
i16 = mybir.dt.int16
FP8NP = ml_dtypes.float8_e4m3


def _w_lo(C):
    alpha = NB / C
    return np.minimum((np.arange(C) * alpha).astype(int), NB - K)


def build_program(C=C_DEF):
    assert C % KC == 0
    ntiles = C // KC
    w_lo = _w_lo(C)
    nc = bacc.Bacc("TRN2", target_bir_lowering=False, debug=False,
                   num_devices=NUM_CORES)
    x_ap = nc.dram_tensor("xpad", [NPAD, D], f32, kind="ExternalInput").ap()
    d16_ap = nc.dram_tensor("dst16", [P, C * 8], i16,
                            kind="ExternalInput").ap()
    oh_ap = nc.dram_tensor("ohd", [P, ntiles * K * TW], fp8,
                           kind="ExternalInput").ap()
    par_ap = nc.dram_tensor("par", [P, C], fp16, kind="ExternalInput").ap()
    cst_ap = nc.dram_tensor("cst", [P, 2], fp16, kind="ExternalInput").ap()
    out_ap = nc.dram_tensor("out", [P, C], f32, kind="ExternalOutput").ap()

    xp = nc.dram_tensor("xp", [NPAIR, 512], fp8).ap()

    with tile.TileContext(nc) as tc:
        with (
            tc.tile_pool(name="xt", bufs=3) as xtp,
            tc.tile_pool(name="pair", bufs=2) as pairp,
            tc.tile_pool(name="xsb", bufs=1) as xsbp,
            tc.tile_pool(name="oh", bufs=2) as ohp,
            tc.tile_pool(name="dq", bufs=2) as dqp,
            tc.tile_pool(name="sq", bufs=2) as sqp,
            tc.tile_pool(name="wide", bufs=1) as widep,
            tc.tile_pool(name="ps", bufs=2, space="PSUM") as psp,
        ):
            # ---- phase C: small loads (first: gathers dep on idx) ----
            idx_sb = widep.tile([P, C * 8], i16, tag="idx")
            nc.sync.dma_start(idx_sb[:], d16_ap[:])
            par_sb = widep.tile([P, C], fp16, tag="par")
            nc.sync.dma_start(par_sb[:], par_ap[:])
            cst = widep.tile([P, 2], fp16, tag="cst")
            nc.sync.dma_start(cst[:], cst_ap[:])

            r2e_w = widep.tile([P, C], fp16, tag="r2e")
            r2o_w = widep.tile([P, C], fp16, tag="r2o")
            m2_w = widep.tile([P, C * 2], fp16, tag="m2")
            tm_w = widep.tile([P, C], fp16, tag="tm")
            mm_w = widep.tile([P, C], fp16, tag="mm")
            outw = widep.tile([P, C], f32, tag="outw")
            m2v = m2_w.rearrange("p (c two) -> p c two", two=2)

            # ---- phase A: build pair table xp + x_sb in one x pass ----
            # x rows viewed as [p, 196 pairs, 2, 129]; xp as [p, 196, 512].
            # Node n = p*392 + c2 (c2 = 2c+t): src block = n%392 = c2,
            # position in block = n//392 = p. So the same tiles also fill
            # x_sb (strided over c2 parity) with no second x read.
            xv = x_ap[:].rearrange("(p c two) d -> p c (two d)", p=P, two=2)
            xpv = xp[:].rearrange("(p c) d -> p c d", p=P)
            x_sb = xsbp.tile([P, NB * DM], fp8, tag="xsb")
            xsv = x_sb.rearrange("p (c f) -> p c f", f=DM)
            CP = NPAIR // P  # 196
            for c0 in range(0, CP, 16):
                cw = min(16, CP - c0)
                t = xtp.tile([P, 4128], f32, tag="xt")
                tv = t.rearrange("p (c d) -> p c d", d=2 * D)
                nc.sync.dma_start(tv[:, :cw, :], xv[:, c0:c0 + cw, :])
                dt = pairp.tile([P, KC, 512], fp8, tag="pair")
                dt16 = dt.bitcast(fp16)
                nc.vector.tensor_copy(dt[:, :cw, 0:DM], tv[:, :cw, 0:DM])
                nc.vector.tensor_copy(dt16[:, :cw, 64:65],
                                      tv[:, :cw, DM:DM + 1])
                nc.vector.tensor_copy(dt[:, :cw, 256:256 + DM],
                                      tv[:, :cw, D:D + DM])
                nc.vector.tensor_copy(dt16[:, :cw, 65:66],
                                      tv[:, :cw, 2 * D - 1:2 * D])
                # scalar-engine HWDGE ring: table writes must not
                # head-of-line block the next x read on the sync ring
                nc.scalar.dma_start(xpv[:, c0:c0 + cw, :], dt[:, :cw, :])
                nc.vector.tensor_copy(xsv[:, 2 * c0:2 * (c0 + cw):2, :],
                                      tv[:, :cw, 0:DM])
                nc.vector.tensor_copy(xsv[:, 2 * c0 + 1:2 * (c0 + cw):2, :],
                                      tv[:, :cw, D:D + DM])

            # ---- phase D: software-pipelined main loop ---------------
            # gathers are paired (4096 idx) to amortize SWDGE overhead
            tiles = {}
            gtiles = {}

            def produce(t_i):
                if t_i % 2 == 0:
                    nt2 = min(2, ntiles - t_i)
                    isl = slice(t_i * P, (t_i + nt2) * P)
                    gt = pairp.tile([P, 2 * KC, 512], fp8, tag="gpair")
                    nc.gpsimd.dma_gather(gt[:, :nt2 * KC, :], xp[:],
                                         idx_sb[:, isl], nt2 * TW, nt2 * TW,
                                         512, single_packet=False)
                    gtiles[t_i // 2] = gt
                osl = slice(t_i * K * TW, (t_i + 1) * K * TW)
                oh = ohp.tile([P, K, TW], fp8, tag="oh")
                nc.sync.dma_start(oh.rearrange("p k w -> p (k w)"),
                                  oh_ap[:, osl])
                ps = psp.tile([P, TW], f32, tag="ps")
                for ch in range(KC):
                    g = t_i * KC + ch
                    w = int(w_lo[g])
                    cs = slice(ch * P, (ch + 1) * P)
                    for k in range(K):
                        nc.tensor.matmul(
                            ps[:, cs], oh[:, k:k + 1, cs],
                            xsv[:, w + k, :],
                            start=(k == 0), stop=(k == K - 1))
                tiles[t_i] = ps

            def consume(t_i):
                ps = tiles.pop(t_i)
                gt = gtiles[t_i // 2]
                h0 = (t_i % 2) * KC
                pt = gt[:, h0:h0 + KC, :]
                slc = slice(t_i * KC, (t_i + 1) * KC)
                sl2 = slice(t_i * KC * 2, (t_i + 1) * KC * 2)
                pt16 = gt.bitcast(fp16)[:, h0:h0 + KC, :]
                sv = ps.rearrange("p (c f) -> p c f", f=DM)
                de = dqp.tile([P, KC, DM], fp16, tag="dq")
                nc.vector.tensor_tensor(out=de[:], in0=sv[:],
                                        in1=pt[:, :, 0:DM],
                                        op=mybir.AluOpType.subtract)
                se = sqp.tile([P, KC, DM], fp16, tag="sq")
                nc.scalar.activation(se[:], de[:],
                                     mybir.ActivationFunctionType.Square)
                with nc.allow_low_precision("r2 fp16 accum: ~1% worst-case"):
                    nc.vector.tensor_reduce(r2e_w[:, slc], se[:],
                                            axis=mybir.AxisListType.X,
                                            op=mybir.AluOpType.add)
                do = dqp.tile([P, KC, DM], fp16, tag="dq")
                nc.vector.tensor_tensor(out=do[:], in0=sv[:],
                                        in1=pt[:, :, 256:256 + DM],
                                        op=mybir.AluOpType.subtract)
                so = sqp.tile([P, KC, DM], fp16, tag="sq")
                nc.scalar.activation(so[:], do[:],
                                     mybir.ActivationFunctionType.Square)
                with nc.allow_low_precision("r2 fp16 accum: ~1% worst-case"):
                    nc.vector.tensor_reduce(r2o_w[:, slc], so[:],
                                            axis=mybir.AxisListType.X,
                                            op=mybir.AluOpType.add)
                nc.scalar.activation(
                    m2_w[:, sl2].rearrange("p (c two) -> p c two", two=2),
                    pt16[:, :, 64:66], mybir.ActivationFunctionType.Copy)

            # epilogue over a column range (split so most of it overlaps
            # the tail of the gather stream):
            # r2 = r2e + par*(r2o - r2e); m likewise; out = m - l*ln(r2+eps)
            def epilogue(c0, c1):
                cs = slice(c0, c1)
                cs2 = slice(c0, c1)
                nc.vector.tensor_tensor(
                    out=r2o_w[:, cs], in0=r2o_w[:, cs], in1=r2e_w[:, cs],
                    op=mybir.AluOpType.subtract)
                nc.vector.tensor_tensor(
                    out=r2o_w[:, cs], in0=r2o_w[:, cs], in1=par_sb[:, cs],
                    op=mybir.AluOpType.mult)
                nc.vector.tensor_tensor(
                    out=r2e_w[:, cs], in0=r2e_w[:, cs], in1=r2o_w[:, cs],
                    op=mybir.AluOpType.add)
                nc.vector.tensor_tensor(
                    out=tm_w[:, cs].unsqueeze(2), in0=m2v[:, cs2, 1:2],
                    in1=m2v[:, cs2, 0:1], op=mybir.AluOpType.subtract)
                nc.vector.tensor_tensor(
                    out=tm_w[:, cs], in0=tm_w[:, cs], in1=par_sb[:, cs],
                    op=mybir.AluOpType.mult)
                nc.vector.tensor_tensor(
                    out=mm_w[:, cs].unsqueeze(2), in0=m2v[:, cs2, 0:1],
                    in1=tm_w[:, cs].unsqueeze(2), op=mybir.AluOpType.add)
                nc.scalar.activation(r2o_w[:, cs], r2e_w[:, cs],
                                     mybir.ActivationFunctionType.Ln,
                                     bias=cst[:, 1:2])
                nc.vector.scalar_tensor_tensor(
                    out=outw[:, cs], in0=r2o_w[:, cs], scalar=cst[:, 0:1],
                    in1=mm_w[:, cs],
                    op0=mybir.AluOpType.mult, op1=mybir.AluOpType.add)
                nc.sync.dma_start(out_ap[:, cs], outw[:, cs])

            ntl = ntiles
            for t_i in range(ntl + 1):
                if t_i < ntl:
                    produce(t_i)
                if t_i > 0:
                    consume(t_i - 1)
                    if t_i == ntl - 1:
                        # all but the last tile's columns: overlaps the
                        # final gather + consume
                        epilogue(0, (ntl - 1) * KC)
            epilogue((ntl - 1) * KC, C)

    nc.compile()
    return nc


_compiled = {}


def _get_compiled(C=C_DEF):
    if C not in _compiled:
        _compiled[C] = build_program(C)
    return _compiled[C]


def _pack_core(src, C):
    """Window-pack edges (sorted by src block) into C chunks of 128 slots.

    Returns slot2edge [C*128] int64 (-1 = pad). Raises OverflowError if C
    is too small.
    """
    w_lo = _w_lo(C)
    order = np.argsort(src % NB, kind="stable")
    blocks = (src[order] % NB).astype(np.int64)
    counts = np.bincount(blocks, minlength=NB)
    slot2edge = np.full(C * P, -1, np.int64)
    c = 0
    fill = 0
    pos = 0
    for b in range(NB):
        n = int(counts[b])
        while n > 0:
            while c < C and not (w_lo[c] <= b < w_lo[c] + K):
                c += 1
                fill = 0
            if c >= C:
                raise OverflowError(C)
            take = min(n, P - fill)
            s0 = c * P + fill
            slot2edge[s0:s0 + take] = order[pos:pos + take]
            pos += take
            n -= take
            fill += take
            if fill == P:
                c += 1
                fill = 0
                if c >= C and n > 0:
                    raise OverflowError(C)
    return slot2edge


def make_in_maps(x, edge_label_index, l_param, C=C_DEF):
    x = np.asarray(x, dtype=np.float32)
    x_pad = np.zeros((NPAD, D), np.float32)
    x_pad[:N] = x
    eli = np.asarray(edge_label_index)
    l = float(np.asarray(l_param).reshape(-1)[0])
    cstv = np.zeros((P, 2), np.float16)
    cstv[:, 0] = -l
    cstv[:, 1] = EPS
    w_lo = _w_lo(C)
    ntiles = C // KC
    chunk_of_slot = np.arange(C * P) // P

    in_maps = []
    slot_maps = []
    for core in range(NUM_CORES):
        sl = slice(core * EC, (core + 1) * EC)
        src = eli[0][sl].astype(np.int64)
        dst = eli[1][sl].astype(np.int64)
        s2e = _pack_core(src, C)
        real = s2e >= 0
        e = s2e[real]
        slots = np.arange(C * P)[real]
        vp = src[e] // NB                            # position in block
        vk = src[e] % NB - w_lo[chunk_of_slot[real]]  # [0, K)
        oh8 = np.zeros((P, ntiles, K, TW), np.uint8)
        oh8[vp, slots // TW, vk, slots % TW] = 0x38  # 1.0 in e4m3
        dstv = np.zeros(C * P, np.int16)
        dstv[real] = (dst[e] // 2).astype(np.int16)
        d16 = np.tile(dstv.reshape(C * P // 16, 16).T, (8, 1))
        par = np.zeros(C * P, np.float16)
        par[real] = (dst[e] % 2).astype(np.float16)
        in_maps.append({
            "xpad": x_pad,
            "dst16": np.ascontiguousarray(d16),
            "ohd": oh8.reshape(P, ntiles * K * TW).view(FP8NP),
            "par": np.ascontiguousarray(par.reshape(C, P).T),
            "cst": cstv,
        })
        slot_maps.append(s2e)
    return in_maps, slot_maps, C


def _unshard(results, slot_maps, C):
    out = np.empty(E, np.float32)
    for core in range(NUM_CORES):
        dev = results[core]["out"]          # [128, C]
        vals = dev.T.ravel()                # slot-major
        s2e = slot_maps[core]
        real = s2e >= 0
        core_out = np.empty(EC, np.float32)
        core_out[s2e[real]] = vals[real]
        out[core * EC:(core + 1) * EC] = core_out
    return out.reshape(E, 1)


def kernel(x, edge_label_index, l_param):
    C = C_DEF
    while True:
        try:
            in_maps, slot_maps, C = make_in_maps(
                x, edge_label_index, l_param, C)
            break
        except OverflowError:
            C += 2 * KC
    nc = _get_compiled(C)
    res = run_bass_kernel_spmd(nc, in_maps, list(range(NUM_CORES)))
    return _unshard(res.results, slot_maps, C)
